# revision 3
# baseline (speedup 1.0000x reference)
"""Causal multi-head attention block (B=4, S=2048, D=768, H=12, Dh=64)
distributed over 8 NeuronCores: core = (batch, head-group), each core
computes its 6 heads end-to-end plus its partial output projection;
host sums the two partials per batch and adds the bias.

Self-contained: hardcodes all shapes; no sibling imports.
"""

import numpy as np

B, S, D = 4, 2048, 768
H, DH = 12, 64
G = 384          # channels per head group (6 heads)
NPAIR = 3        # head pairs per core
NSC = 4          # 512-wide query windows
W = 512
NST = 16         # 128-row s-tiles
NDC = 6          # 128-row D chunks

_PROGRAM = None
PROFILE = False
PROFILE_DIR = None
LAST_RESULT = None


def _split_waits(nc, max_waits=1, max_updates=1):
    """This container's walrus rejects instructions carrying more than one
    semaphore wait/update ("Too many sync wait commands").  Move excess
    waits onto NoOps inserted before the owning instruction (same engine)
    and excess updates onto NoOps inserted after."""
    import concourse.mybir as mybir

    counter = [0]

    def nop(engine, waits, updates):
        counter[0] += 1
        n = mybir.InstNoOp(name=f"wsplit_nop_{counter[0]}", ins=[], outs=[])
        n.engine = engine
        n.sync_info = mybir.SyncInfo(on_wait=waits, on_update=updates)
        return n

    for bb in nc.main_func.blocks:
        out = []
        changed = False
        for ins in bb.instructions:
            si = ins.sync_info
            waits = list(si.on_wait) if si and si.on_wait else []
            updates = list(si.on_update) if si and si.on_update else []
            pre, post = [], []
            if len(waits) > max_waits:
                keep = waits[:max_waits - 1] if max_waits > 1 else []
                rest = waits[len(keep):]
                while rest:
                    chunk, rest = rest[:max_waits], rest[max_waits:]
                    pre.append(chunk)
                waits = keep
                changed = True
            if len(updates) > max_updates:
                rest = updates[max_updates:]
                updates = updates[:max_updates]
                while rest:
                    chunk, rest = rest[:max_updates], rest[max_updates:]
                    post.append(chunk)
                changed = True
            if pre or post:
                ins.sync_info = mybir.SyncInfo(
                    on_wait=waits, on_update=updates)
            for w in pre:
                out.append(nop(ins.engine, w, []))
            out.append(ins)
            for u in post:
                out.append(nop(ins.engine, [], u))
        if changed:
            bb.instructions = out


def _install_profile_hooks():
    """Dev-only (PROFILE=True): register the NTFF profile hook that the
    agent image's antenv lacks, and stub out the artifact upload."""
    import sys
    import types

    try:
        from antenv.axon_hooks import get_axon_ntff_profile_hook  # noqa: F401
    except ImportError:
        import antenv
        from trn_agent_boot import trn_boot

        hook = trn_boot._ntff_profile_via_ctypes("/opt/axon/libaxon_pjrt.so")
        mod = types.ModuleType("antenv.axon_hooks")
        mod._hook = hook
        mod.get_axon_ntff_profile_hook = lambda: mod._hook
        mod.set_axon_ntff_profile_hook = lambda h: setattr(mod, "_hook", h)
        sys.modules["antenv.axon_hooks"] = mod
        antenv.axon_hooks = mod

    from concourse import bass_utils

    bass_utils.upload_artifacts = lambda tmpdir: "local://" + tmpdir


def _build_program():
    import concourse.bass as bass
    import concourse.mybir as mybir
    import concourse.tile as tile

    f16 = mybir.dt.float16
    f32 = mybir.dt.float32

    nc = bass.Bass()
    xt_d = nc.declare_dram_parameter("xt", [D, S], f16, isOutput=False)
    wq_d = nc.declare_dram_parameter("wq", [D, G], f16, isOutput=False)
    wk_d = nc.declare_dram_parameter("wk", [D, G], f16, isOutput=False)
    wv_d = nc.declare_dram_parameter("wv", [D, G], f16, isOutput=False)
    wo_d = nc.declare_dram_parameter("wo", [G, D], f16, isOutput=False)
    mk_d = nc.declare_dram_parameter("mk", [128, 128], f16, isOutput=False)
    y_d = nc.declare_dram_parameter("y", [S, D], f32, isOutput=True)

    with tile.TileContext(nc) as tc:
        with (
            tc.tile_pool(name="const", bufs=1) as const,
            tc.tile_pool(name="work", bufs=3) as work,
            tc.tile_pool(name="outp", bufs=3) as outp,
            tc.tile_pool(name="ps", bufs=2, space="PSUM") as ps,
        ):
            # ---- persistent SBUF tiles ----
            xt = [const.tile([128, S], f16, name=f"xt{i}", tag=f"xt{i}")
                  for i in range(NDC)]
            wq = [const.tile([128, G], f16, name=f"wq{i}", tag=f"wq{i}")
                  for i in range(NDC)]
            wk = [const.tile([128, G], f16, name=f"wk{i}", tag=f"wk{i}")
                  for i in range(NDC)]
            wv = [const.tile([128, G], f16, name=f"wv{i}", tag=f"wv{i}")
                  for i in range(NDC)]
            wo = [const.tile([128, D], f16, name=f"wo{i}", tag=f"wo{i}")
                  for i in range(3)]
            qt = [const.tile([128, S], f16, name=f"qt{p}", tag=f"qt{p}")
                  for p in range(NPAIR)]
            kt = [const.tile([128, S], f16, name=f"kt{p}", tag=f"kt{p}")
                  for p in range(NPAIR)]
            vt = [const.tile([128, G], f16, name=f"vt{t}", tag=f"vt{t}")
                  for t in range(NST)]
            gt = [const.tile([128, S], f16, name=f"gt{p}", tag=f"gt{p}")
                  for p in range(NPAIR)]
            mk = const.tile([128, 128], f16, name="mk", tag="mk")
            ones = const.tile([128, DH], f16, name="ones", tag="ones")

            # ---- input DMAs (weights first so the first projection
            # group can start as soon as xt chunk 0 lands) ----
            nc.sync.dma_start(out=mk, in_=mk_d[:, :])
            for i in range(NDC):
                nc.sync.dma_start(out=wq[i], in_=wq_d[128 * i:128 * (i + 1), :])
                nc.gpsimd.dma_start(out=wk[i], in_=wk_d[128 * i:128 * (i + 1), :])
                nc.gpsimd.dma_start(out=wv[i], in_=wv_d[128 * i:128 * (i + 1), :])
            for i in range(NDC):
                nc.sync.dma_start(out=xt[i][:, 0:S // 2],
                                  in_=xt_d[128 * i:128 * (i + 1), 0:S // 2])
                nc.gpsimd.dma_start(out=xt[i][:, S // 2:S],
                                  in_=xt_d[128 * i:128 * (i + 1), S // 2:S])
            for i in range(3):
                nc.gpsimd.dma_start(out=wo[i], in_=wo_d[128 * i:128 * (i + 1), :])
            nc.vector.memset(ones, 1.0)

            def act_recip(out, in_):
                # ScalarE table reciprocal (~1e-5 rel err on [1e-2, 1e7],
                # verified on HW) -- keeps the softmax divide off the DVE
                # and off the inter-window critical path.
                eng = nc.scalar
                ins_ = [eng.lower_ap(in_[:, :]),
                        mybir.ImmediateValue(dtype=mybir.dt.float32, value=0.0),
                        mybir.ImmediateValue(dtype=mybir.dt.float32, value=1.0),
                        mybir.ImmediateValue(dtype=mybir.dt.float32, value=0.0)]
                eng.add_instruction(mybir.InstActivation(
                    name=nc.get_next_instruction_name(),
                    func=mybir.ActivationFunctionType.Reciprocal,
                    ins=ins_, outs=[eng.lower_ap(out[:, :])]))

            def proj_qk_unit(pair, sc):
                qp = ps.tile([128, W], f32, name=f"qp{pair}_{sc}",
                             tag="sc", bufs=2)
                for dc in range(NDC):
                    nc.tensor.matmul(
                        qp,
                        wq[dc][:, 128 * pair:128 * (pair + 1)],
                        xt[dc][:, W * sc:W * (sc + 1)],
                        start=(dc == 0), stop=(dc == NDC - 1))
                nc.vector.tensor_copy(
                    out=qt[pair][:, W * sc:W * (sc + 1)], in_=qp)
                kp = ps.tile([128, W], f32, name=f"kp{pair}_{sc}",
                             tag="sc", bufs=2)
                for dc in range(NDC):
                    nc.tensor.matmul(
                        kp,
                        wk[dc][:, 128 * pair:128 * (pair + 1)],
                        xt[dc][:, W * sc:W * (sc + 1)],
                        start=(dc == 0), stop=(dc == NDC - 1))
                nc.vector.tensor_copy(
                    out=kt[pair][:, W * sc:W * (sc + 1)], in_=kp)

            def proj_v(st):
                vp = ps.tile([128, G], f32, name=f"vp{st}", tag="sc", bufs=2)
                for dc in range(NDC):
                    nc.tensor.matmul(
                        vp,
                        xt[dc][:, 128 * st:128 * (st + 1)],
                        wv[dc],
                        start=(dc == 0), stop=(dc == NDC - 1))
                nc.vector.tensor_copy(out=vt[st], in_=vp)

            def outproj(st):
                o0 = ps.tile([128, G], f32, name=f"o0_{st}", tag="apv", bufs=2)
                for cc in range(3):
                    nc.tensor.matmul(
                        o0,
                        gt[cc][:, 128 * st:128 * (st + 1)],
                        wo[cc][:, 0:G],
                        start=(cc == 0), stop=(cc == 2))
                o1 = ps.tile([128, G], f32, name=f"o1_{st}", tag="adn", bufs=2)
                for cc in range(3):
                    nc.tensor.matmul(
                        o1,
                        gt[cc][:, 128 * st:128 * (st + 1)],
                        wo[cc][:, G:D],
                        start=(cc == 0), stop=(cc == 2))
                ob = outp.tile([128, D], f32, name=f"ob{st}", tag="ob", bufs=4)
                nc.vector.tensor_copy(out=ob[:, 0:G], in_=o0)
                nc.vector.tensor_copy(out=ob[:, G:D], in_=o1)
                eng = nc.sync if st % 2 == 0 else nc.gpsimd
                eng.dma_start(
                    out=y_d[128 * st:128 * (st + 1), :], in_=ob)

            class PairAttention:
                """Per-pair attention with a software pipeline that crosses
                window boundaries: pv/dn of group g are issued after the
                scores of group g+1 (even when g+1 is in the next query
                window), so neither the PE nor ACT drains at boundaries."""

                def __init__(self, pair):
                    self.pair = pair
                    self.prev = None

                def scores_exp(self, sc, jb):
                    pair = self.pair
                    col0 = max(0, 128 * jb - W * sc)
                    diag = jb >= 4 * sc
                    sct = ps.tile([128, 1024], f32, name=f"sc{pair}_{sc}_{jb}",
                                  tag="sc", bufs=2)
                    # 64x64 PE tiling: each head's 128-key block splits into
                    # two M=64 col-tiles; with head0 on contraction rows 0:64
                    # and head1 on rows 64:128 all four matmuls run
                    # concurrently at full stream rate (auto tile_position
                    # from lhsT/out base partitions).  Row groups write
                    # different PSUM banks (head0 cols<W, head1 cols>=W).
                    k0 = 128 * jb
                    q = qt[pair]
                    qs0 = q[0:64, W * sc + col0:W * (sc + 1)]
                    qs1 = q[64:128, W * sc + col0:W * (sc + 1)]
                    nc.tensor.matmul(
                        sct[0:64, col0:W],
                        kt[pair][0:64, k0:k0 + 64], qs0,
                        start=True, stop=True)
                    nc.tensor.matmul(
                        sct[64:128, col0:W],
                        kt[pair][0:64, k0 + 64:k0 + 128], qs0,
                        start=True, stop=True)
                    nc.tensor.matmul(
                        sct[0:64, W:2 * W - col0],
                        kt[pair][64:128, k0:k0 + 64], qs1,
                        start=True, stop=True)
                    nc.tensor.matmul(
                        sct[64:128, W:2 * W - col0],
                        kt[pair][64:128, k0 + 64:k0 + 128], qs1,
                        start=True, stop=True)
                    ex = work.tile([128, 1024], f16, name=f"ex{pair}_{sc}_{jb}",
                                   tag="exp", bufs=6)
                    nc.scalar.activation(
                        out=ex[:, col0:2 * W - col0],
                        in_=sct[:, col0:2 * W - col0],
                        func=mybir.ActivationFunctionType.Exp, scale=0.125)
                    if diag:  # zero the j>i triangle of the diagonal block
                        nc.vector.tensor_mul(
                            ex[:, col0:col0 + 128], ex[:, col0:col0 + 128], mk)
                        nc.vector.tensor_mul(
                            ex[:, W:W + 128], ex[:, W:W + 128], mk)
                    return ex

                def pv_dn(self, state):
                    pair = self.pair
                    pv, dn, sc, jb, ex = state
                    col0 = max(0, 128 * jb - W * sc)
                    first, last = (jb == 0), (jb == 4 * sc + 3)
                    nc.tensor.matmul(
                        pv[0:64, col0:W],
                        vt[jb][:, 128 * pair:128 * pair + 64],
                        ex[:, col0:W],
                        start=first, stop=last)
                    nc.tensor.matmul(
                        pv[64:128, col0:W],
                        vt[jb][:, 128 * pair + 64:128 * (pair + 1)],
                        ex[:, W:2 * W - col0],
                        start=first, stop=last)
                    nc.tensor.matmul(
                        dn[0:64, col0:W],
                        ones,
                        ex[:, col0:W],
                        start=first, stop=last)
                    nc.tensor.matmul(
                        dn[64:128, col0:W],
                        ones,
                        ex[:, W:2 * W - col0],
                        start=first, stop=last)
                    if last:  # window complete: normalize into gt
                        rc = work.tile([128, W], f32, name=f"rc{pair}_{sc}",
                                       tag="rc", bufs=3)
                        act_recip(rc, dn)
                        nc.vector.tensor_mul(
                            gt[pair][:, W * sc:W * (sc + 1)], pv, rc)

                def window(self, sc, filler_tick):
                    pair = self.pair
                    pv = ps.tile([128, W], f32, name=f"pv{pair}_{sc}",
                                 tag="apv", bufs=2)
                    dn = ps.tile([128, W], f32, name=f"dn{pair}_{sc}",
                                 tag="adn", bufs=2)
                    for jb in range(4 * sc + 4):
                        ex = self.scores_exp(sc, jb)
                        if self.prev is not None:
                            self.pv_dn(self.prev)
                        self.prev = (pv, dn, sc, jb, ex)
                        filler_tick()

                def flush(self):
                    if self.prev is not None:
                        self.pv_dn(self.prev)
                        self.prev = None

            def attention(pair_obj, sc, filler_tick):
                pair_obj.window(sc, filler_tick)

            # ---- orchestration ----
            # Only the first window's q/k projection precedes attention;
            # later windows' projections ride inside earlier windows, so
            # the exp stream starts as soon as xt lands (~24us).
            proj_qk_unit(0, 0)
            for st in range(4):
                proj_v(st)

            def make_tick(queue, period):
                ticks = [0]

                def tick():
                    ticks[0] += 1
                    if ticks[0] % period == 0 and queue:
                        queue.pop(0)()
                return tick

            pa0 = PairAttention(0)
            for sc in range(NSC):
                q = []
                if sc + 1 < NSC:
                    q.append(lambda s=sc + 1: proj_qk_unit(0, s))
                    q += [(lambda st=st: proj_v(st))
                          for st in range(4 * sc + 4, 4 * sc + 8)]
                    period = 1 if sc == 0 else 2
                else:
                    q = [(lambda s=s: proj_qk_unit(1, s)) for s in range(NSC)]
                    period = 4
                attention(pa0, sc, make_tick(q, period))
                while q:
                    q.pop(0)()
            pa0.flush()
            pa1 = PairAttention(1)
            for sc in range(NSC):
                if sc < NSC - 1:
                    attention(pa1, sc, lambda: None)
                else:
                    q = [(lambda s=s: proj_qk_unit(2, s)) for s in range(NSC)]
                    attention(pa1, sc, make_tick(q, 4))
                    while q:
                        q.pop(0)()
            pa1.flush()
            pa2 = PairAttention(2)
            emitted = [0]
            for sc in range(NSC):
                allowed = max(0, 4 * sc)
                ticks = [0]

                def tick(allowed=allowed, ticks=ticks):
                    ticks[0] += 1
                    if ticks[0] % 3 == 0 and emitted[0] < min(allowed, 12):
                        outproj(emitted[0])
                        emitted[0] += 1

                attention(pa2, sc, tick)
            pa2.flush()
            for st in range(emitted[0], NST):
                outproj(st)

    _split_waits(nc)
    return nc


def _get_program():
    global _PROGRAM
    if _PROGRAM is None:
        _PROGRAM = _build_program()
    return _PROGRAM


def kernel(x, Wq, Wk, Wv, Wo, bo):
    global LAST_RESULT
    from concourse.bass_utils import run_bass_kernel_spmd

    x = np.asarray(x, np.float32)
    Wq = np.asarray(Wq, np.float32)
    Wk = np.asarray(Wk, np.float32)
    Wv = np.asarray(Wv, np.float32)
    Wo = np.asarray(Wo, np.float32)
    bo = np.asarray(bo, np.float32)

    tri = np.tril(np.ones((128, 128), np.float32)).T  # 1 where j<=i
    mk = tri.astype(np.float16)

    in_maps = []
    for c in range(8):
        b, g = divmod(c, 2)
        hs = slice(G * g, G * (g + 1))
        in_maps.append({
            "xt": np.ascontiguousarray(x[b].T).astype(np.float16),
            "wq": np.ascontiguousarray(Wq[hs, :].T).astype(np.float16),
            "wk": np.ascontiguousarray(Wk[hs, :].T).astype(np.float16),
            "wv": np.ascontiguousarray(Wv[hs, :].T).astype(np.float16),
            "wo": np.ascontiguousarray(Wo[:, hs].T).astype(np.float16),
            "mk": mk,
        })

    if PROFILE:
        _install_profile_hooks()
    nc = _get_program()
    res = run_bass_kernel_spmd(nc, in_maps, core_ids=list(range(8)),
                               trace=PROFILE, tmpdir=PROFILE_DIR)
    LAST_RESULT = res
    parts = [res.results[c]["y"] for c in range(8)]
    out = np.stack([parts[2 * b] + parts[2 * b + 1] + bo for b in range(B)])
    return out.astype(np.float32)



# revision 18
# speedup vs baseline: 1.1422x; 1.1422x over previous
"""Causal multi-head attention block (B=4, S=2048, D=768, H=12, Dh=64)
distributed over 8 NeuronCores: core = (batch, head-group), each core
computes its 6 heads end-to-end plus its partial output projection;
host sums the two partials per batch and adds the bias.

Self-contained: hardcodes all shapes; no sibling imports.
"""

import numpy as np

B, S, D = 4, 2048, 768
H, DH = 12, 64
G = 384          # channels per head group (6 heads)
NPAIR = 3        # head pairs per core
NSC = 4          # 512-wide query windows
W = 512
NST = 16         # 128-row s-tiles
NDC = 6          # 128-row D chunks

# packed-weights layout (per-partition column offsets in the wts tile)
MK0 = 0
QB = [128, 3968, 5504]       # wq pair-column blocks (6 chunks x 128)
KB = [896, 4736, 6272]       # wk pair-column blocks
WV0 = 1664                   # full wv (6 chunks x 384)
WO0 = 7040                   # wo (3 chunks x 768)
WX = 9344

_PROGRAM = None
PROFILE = False
PROFILE_DIR = None
LAST_RESULT = None


def _split_waits(nc, max_waits=1, max_updates=1):
    """This container's walrus rejects instructions carrying more than one
    semaphore wait/update ("Too many sync wait commands").  Move excess
    waits onto NoOps inserted before the owning instruction (same engine)
    and excess updates onto NoOps inserted after."""
    import concourse.mybir as mybir

    counter = [0]

    def nop(engine, waits, updates):
        counter[0] += 1
        n = mybir.InstNoOp(name=f"wsplit_nop_{counter[0]}", ins=[], outs=[])
        n.engine = engine
        n.sync_info = mybir.SyncInfo(on_wait=waits, on_update=updates)
        return n

    for bb in nc.main_func.blocks:
        out = []
        changed = False
        for ins in bb.instructions:
            si = ins.sync_info
            waits = list(si.on_wait) if si and si.on_wait else []
            updates = list(si.on_update) if si and si.on_update else []
            pre, post = [], []
            if len(waits) > max_waits:
                keep = waits[:max_waits - 1] if max_waits > 1 else []
                rest = waits[len(keep):]
                while rest:
                    chunk, rest = rest[:max_waits], rest[max_waits:]
                    pre.append(chunk)
                waits = keep
                changed = True
            if len(updates) > max_updates:
                rest = updates[max_updates:]
                updates = updates[:max_updates]
                while rest:
                    chunk, rest = rest[:max_updates], rest[max_updates:]
                    post.append(chunk)
                changed = True
            if pre or post:
                ins.sync_info = mybir.SyncInfo(
                    on_wait=waits, on_update=updates)
            for w in pre:
                out.append(nop(ins.engine, w, []))
            out.append(ins)
            for u in post:
                out.append(nop(ins.engine, [], u))
        if changed:
            bb.instructions = out


def _install_profile_hooks():
    """Dev-only (PROFILE=True): register the NTFF profile hook that the
    agent image's antenv lacks, and stub out the artifact upload."""
    import sys
    import types

    try:
        from antenv.axon_hooks import get_axon_ntff_profile_hook  # noqa: F401
    except ImportError:
        import antenv
        from trn_agent_boot import trn_boot

        hook = trn_boot._ntff_profile_via_ctypes("/opt/axon/libaxon_pjrt.so")
        mod = types.ModuleType("antenv.axon_hooks")
        mod._hook = hook
        mod.get_axon_ntff_profile_hook = lambda: mod._hook
        mod.set_axon_ntff_profile_hook = lambda h: setattr(mod, "_hook", h)
        sys.modules["antenv.axon_hooks"] = mod
        antenv.axon_hooks = mod

    from concourse import bass_utils

    bass_utils.upload_artifacts = lambda tmpdir: "local://" + tmpdir


def _build_program():
    import concourse.bass as bass
    import concourse.mybir as mybir
    import concourse.tile as tile

    f16 = mybir.dt.float16
    f32 = mybir.dt.float32

    nc = bass.Bass()
    # xt is block-major: [128, window, chunk*512] so each window's slice is
    # one contiguous 6KB-per-partition DMA.
    xt_d = nc.declare_dram_parameter("xt", [128, NSC, NDC * W], f16,
                                     isOutput=False)
    wts_d = nc.declare_dram_parameter("wts", [128, WX], f16, isOutput=False)
    y_d = nc.declare_dram_parameter("y", [S, D], f16, isOutput=True)

    with tile.TileContext(nc) as tc:
        with (
            tc.tile_pool(name="const", bufs=1) as const,
            tc.tile_pool(name="work", bufs=3) as work,
            tc.tile_pool(name="outp", bufs=3) as outp,
            tc.tile_pool(name="ps", bufs=2, space="PSUM") as ps,
        ):
            # ---- persistent SBUF tiles ----
            wts = const.tile([128, WX], f16, name="wts", tag="wts")
            xt = const.tile([128, NSC, NDC * W], f16, name="xt", tag="xt")

            def xv(sc, dc, c0, c1):
                # chunk dc, absolute cols W*sc+c0 .. W*sc+c1
                return xt[:, sc, W * dc + c0:W * dc + c1]
            qt = [const.tile([128, S], f16, name=f"qt{p}", tag=f"qt{p}")
                  for p in range(NPAIR)]
            kt = [const.tile([128, S], f16, name=f"kt{p}", tag=f"kt{p}")
                  for p in range(NPAIR)]
            gt = [const.tile([128, S], f16, name=f"gt{p}", tag=f"gt{p}")
                  for p in range(NPAIR)]
            # vt[st]: per head h the 128 lhsT columns [v_h (64) | ones (64)]
            # so one matmul per head accumulates attn@V on out partitions
            # 0:64 and the softmax denominator (replicated) on 64:128.
            vt = [const.tile([128, 2 * NPAIR, 128], f16, name=f"vt{t}",
                             tag=f"vt{t}") for t in range(NST)]

            mkv = wts[:, MK0:MK0 + 128]

            def wqv(pair, dc):
                return wts[:, QB[pair] + 128 * dc:QB[pair] + 128 * (dc + 1)]

            def wkv(pair, dc):
                return wts[:, KB[pair] + 128 * dc:KB[pair] + 128 * (dc + 1)]

            def wvv(dc):
                return wts[:, WV0 + 384 * dc:WV0 + 384 * (dc + 1)]

            def wov(cc, half):
                b = WO0 + 768 * cc + 384 * half
                return wts[:, b:b + 384]

            # ---- input DMAs, need-ordered; both queues share one HBM
            # stream so the first-window deps (mk+pair0 qk, xt cols 0:512,
            # wv) go first and the rest rides behind compute ----
            nc.sync.dma_start(out=wts[:, 0:WV0], in_=wts_d[:, 0:WV0])
            nc.gpsimd.dma_start(out=wts[:, WV0:QB[1]],
                                in_=wts_d[:, WV0:QB[1]])
            nc.sync.dma_start(out=xt[:, 0, :], in_=xt_d[:, 0, :])
            nc.gpsimd.dma_start(out=wts[:, QB[1]:WO0],
                                in_=wts_d[:, QB[1]:WO0])
            nc.sync.dma_start(out=xt[:, 1, :], in_=xt_d[:, 1, :])
            nc.gpsimd.dma_start(out=xt[:, 2, :], in_=xt_d[:, 2, :])
            nc.sync.dma_start(out=xt[:, 3, :], in_=xt_d[:, 3, :])
            nc.gpsimd.dma_start(out=wts[:, WO0:WX], in_=wts_d[:, WO0:WX])

            for st in range(NST):
                nc.vector.memset(vt[st][:, :, 64:128], 1.0)

            def proj_qk_unit(pair, sc):
                qp = ps.tile([128, W], f32, name=f"qp{pair}_{sc}",
                             tag="sc", bufs=2)
                for dc in range(NDC):
                    nc.tensor.matmul(
                        qp, wqv(pair, dc), xv(sc, dc, 0, W),
                        start=(dc == 0), stop=(dc == NDC - 1))
                nc.vector.tensor_copy(
                    out=qt[pair][:, W * sc:W * (sc + 1)], in_=qp)
                kp = ps.tile([128, W], f32, name=f"kp{pair}_{sc}",
                             tag="sc", bufs=2)
                for dc in range(NDC):
                    nc.tensor.matmul(
                        kp, wkv(pair, dc), xv(sc, dc, 0, W),
                        start=(dc == 0), stop=(dc == NDC - 1))
                nc.vector.tensor_copy(
                    out=kt[pair][:, W * sc:W * (sc + 1)], in_=kp)

            def proj_v(st):
                vp = ps.tile([128, 2 * NPAIR, 64], f32, name=f"vp{st}",
                             tag="sc", bufs=2)
                for dc in range(NDC):
                    nc.tensor.matmul(
                        vp,
                        xv(st // 4, dc, 128 * (st % 4), 128 * (st % 4 + 1)),
                        wvv(dc),
                        start=(dc == 0), stop=(dc == NDC - 1))
                nc.vector.tensor_copy(out=vt[st][:, :, 0:64], in_=vp)

            def outproj(st):
                o0 = ps.tile([128, G], f32, name=f"o0_{st}", tag="sc",
                             bufs=2)
                for cc in range(3):
                    nc.tensor.matmul(
                        o0, gt[cc][:, 128 * st:128 * (st + 1)], wov(cc, 0),
                        start=(cc == 0), stop=(cc == 2))
                o1 = ps.tile([128, G], f32, name=f"o1_{st}", tag="sc",
                             bufs=2)
                for cc in range(3):
                    nc.tensor.matmul(
                        o1, gt[cc][:, 128 * st:128 * (st + 1)], wov(cc, 1),
                        start=(cc == 0), stop=(cc == 2))
                ob = outp.tile([128, D], f16, name=f"ob{st}", tag="ob",
                               bufs=4)
                nc.vector.tensor_copy(out=ob[:, 0:G], in_=o0)
                nc.scalar.activation(
                    out=ob[:, G:D], in_=o1,
                    func=mybir.ActivationFunctionType.Copy)
                eng = nc.sync if st % 2 == 0 else nc.gpsimd
                eng.dma_start(
                    out=y_d[128 * st:128 * (st + 1), :], in_=ob)

            # ---- attention: one global software pipeline over all
            # (sc, pair, jb) blocks so neither the PE nor ACT drains at
            # window or pair boundaries ----
            def scores_exp(pair, sc, jb):
                col0 = max(0, 128 * jb - W * sc)
                sct = ps.tile([128, 1024], f32, name=f"sc{pair}_{sc}_{jb}",
                              tag="sc", bufs=2)
                nc.tensor.matmul(
                    sct[:, col0:W],
                    kt[pair][0:64, 128 * jb:128 * (jb + 1)],
                    qt[pair][0:64, W * sc + col0:W * (sc + 1)],
                    start=True, stop=True)
                nc.tensor.matmul(
                    sct[:, W:2 * W - col0],
                    kt[pair][64:128, 128 * jb:128 * (jb + 1)],
                    qt[pair][64:128, W * sc + col0:W * (sc + 1)],
                    start=True, stop=True)
                ex = work.tile([128, 1024], f16, name=f"ex{pair}_{sc}_{jb}",
                               tag="exp", bufs=6)
                nc.scalar.activation(
                    out=ex[:, col0:2 * W - col0],
                    in_=sct[:, col0:2 * W - col0],
                    func=mybir.ActivationFunctionType.Exp, scale=0.125)
                if jb >= 4 * sc:  # zero the j>i triangle of the diag block
                    nc.gpsimd.tensor_mul(
                        ex[:, col0:col0 + 128], ex[:, col0:col0 + 128], mkv)
                    nc.gpsimd.tensor_mul(
                        ex[:, W:W + 128], ex[:, W:W + 128], mkv)
                return ex

            def pv_dn(state):
                pv0, pv1, pair, sc, jb, ex = state
                col0 = max(0, 128 * jb - W * sc)
                first, last = (jb == 0), (jb == 4 * sc + 3)
                nc.tensor.matmul(
                    pv0[:, col0:W], vt[jb][:, 2 * pair, :],
                    ex[:, col0:W], start=first, stop=last)
                nc.tensor.matmul(
                    pv1[:, col0:W], vt[jb][:, 2 * pair + 1, :],
                    ex[:, W:2 * W - col0], start=first, stop=last)
                if last:  # window complete: normalize into gt
                    # 1/dn as exp(-ln(dn)) on ScalarE: ln+exp+copy share one
                    # activation table set, so no table thrash, and the DVE
                    # FIFO stays clear of the slow iterative reciprocal.
                    cols = slice(W * sc, W * (sc + 1))
                    dnb = work.tile([128, W], f32, name=f"dn{pair}_{sc}",
                                    tag="dnb", bufs=2)
                    nc.vector.tensor_copy(out=dnb[0:64, :],
                                          in_=pv0[64:128, :])
                    nc.vector.tensor_copy(out=dnb[64:128, :],
                                          in_=pv1[64:128, :])
                    rc = work.tile([128, W], f32, name=f"rc{pair}_{sc}",
                                   tag="rc", bufs=2)
                    nc.scalar.activation(
                        out=rc, in_=dnb,
                        func=mybir.ActivationFunctionType.Ln)
                    nc.scalar.activation(
                        out=rc, in_=rc,
                        func=mybir.ActivationFunctionType.Exp, scale=-1.0)
                    nc.vector.tensor_mul(
                        gt[pair][0:64, cols], pv0[0:64, :], rc[0:64, :])
                    nc.vector.tensor_mul(
                        gt[pair][64:128, cols], pv1[0:64, :], rc[64:128, :])

            # static filler plan: emit projection / out-proj units after
            # given global block indices (they're needed ~one round later
            # than emitted; DMA arrival order matches)
            fillers = {
                0: [lambda: proj_v(1)],
                1: [lambda: proj_qk_unit(1, 0)],
                2: [lambda: proj_v(2)],
                3: [lambda: proj_v(3)],
                4: [lambda: proj_qk_unit(2, 0)],
                6: [lambda: proj_qk_unit(0, 1)],
                8: [lambda: proj_qk_unit(1, 1)],
                10: [lambda: proj_qk_unit(2, 1)],
                12: [lambda: proj_v(4)],
                14: [lambda: proj_v(5)],
                16: [lambda: proj_v(6)],
                18: [lambda: proj_v(7)],
                20: [lambda: outproj(0)],
                22: [lambda: outproj(1)],
                24: [lambda: outproj(2)],
                26: [lambda: outproj(3)],
                28: [lambda: proj_qk_unit(0, 2)],
                30: [lambda: proj_qk_unit(1, 2)],
                32: [lambda: proj_qk_unit(2, 2)],
                36: [lambda: proj_v(8)],
                38: [lambda: proj_v(9)],
                40: [lambda: proj_v(10)],
                42: [lambda: proj_v(11)],
                45: [lambda: outproj(4)],
                48: [lambda: outproj(5)],
                51: [lambda: outproj(6)],
                54: [lambda: outproj(7)],
                57: [lambda: proj_qk_unit(0, 3)],
                60: [lambda: proj_qk_unit(1, 3)],
                63: [lambda: proj_qk_unit(2, 3)],
                66: [lambda: proj_v(12)],
                68: [lambda: proj_v(13)],
                70: [lambda: proj_v(14)],
                72: [lambda: proj_v(15)],
                75: [lambda: outproj(8)],
                79: [lambda: outproj(9)],
                83: [lambda: outproj(10)],
                87: [lambda: outproj(11)],
            }

            proj_qk_unit(0, 0)
            proj_v(0)

            prev = [None]

            def block(pair, sc, jb, pv0, pv1):
                ex = scores_exp(pair, sc, jb)
                if prev[0] is not None:
                    pv_dn(prev[0])
                prev[0] = (pv0, pv1, pair, sc, jb, ex)

            g = 0
            for sc in range(NSC):
                for pair in range(NPAIR):
                    pv0 = ps.tile([128, W], f32, name=f"pv0_{pair}_{sc}",
                                  tag="apv", bufs=2)
                    pv1 = ps.tile([128, W], f32, name=f"pv1_{pair}_{sc}",
                                  tag="adn", bufs=2)
                    for jb in range(4 * sc + 4):
                        block(pair, sc, jb, pv0, pv1)
                        for fn in fillers.get(g, ()):
                            fn()
                        g += 1
            pv_dn(prev[0])
            for st in range(12, NST):
                outproj(st)

    _split_waits(nc)
    return nc


def _get_program():
    global _PROGRAM
    if _PROGRAM is None:
        _PROGRAM = _build_program()
    return _PROGRAM


def _pack_chunks(wT, width):
    # [768, width] -> [128, 6*width] with chunk-major per-partition layout
    return np.ascontiguousarray(
        wT.reshape(NDC, 128, width).transpose(1, 0, 2).reshape(128, -1))


def kernel(x, Wq, Wk, Wv, Wo, bo):
    global LAST_RESULT
    from concourse.bass_utils import run_bass_kernel_spmd

    x = np.asarray(x, np.float32)
    Wq = np.asarray(Wq, np.float32)
    Wk = np.asarray(Wk, np.float32)
    Wv = np.asarray(Wv, np.float32)
    Wo = np.asarray(Wo, np.float32)
    bo = np.asarray(bo, np.float32)

    tri = np.tril(np.ones((128, 128), np.float32)).T  # 1 where j<=i
    mk = tri.astype(np.float16)

    in_maps = []
    for c in range(8):
        b, gi = divmod(c, 2)
        hs = slice(G * gi, G * (gi + 1))
        xt = np.ascontiguousarray(x[b].T).astype(np.float16)
        xt3 = xt.reshape(NDC, 128, NSC, W).transpose(1, 2, 0, 3).reshape(
            128, NSC, NDC * W)
        wqT = Wq[hs, :].T.astype(np.float16)   # [768, 384]
        wkT = Wk[hs, :].T.astype(np.float16)
        wvT = Wv[hs, :].T.astype(np.float16)
        woT = Wo[:, hs].T.astype(np.float16)   # [384, 768]

        parts = [mk]
        for pr in range(NPAIR):
            if pr == 1:
                parts.append(_pack_chunks(wvT, G))
            parts.append(_pack_chunks(
                np.ascontiguousarray(wqT[:, 128 * pr:128 * (pr + 1)]), 128))
            parts.append(_pack_chunks(
                np.ascontiguousarray(wkT[:, 128 * pr:128 * (pr + 1)]), 128))
        parts.append(np.ascontiguousarray(
            woT.reshape(3, 128, D).transpose(1, 0, 2).reshape(128, -1)))
        wts = np.concatenate(parts, axis=1)
        assert wts.shape == (128, WX), wts.shape

        in_maps.append({
            "xt": np.ascontiguousarray(xt3),
            "wts": np.ascontiguousarray(wts),
        })

    if PROFILE:
        _install_profile_hooks()
    nc = _get_program()
    res = run_bass_kernel_spmd(nc, in_maps, core_ids=list(range(8)),
                               trace=PROFILE, tmpdir=PROFILE_DIR)
    LAST_RESULT = res
    parts = [res.results[c]["y"] for c in range(8)]
    out = np.stack([parts[2 * b].astype(np.float32)
                    + parts[2 * b + 1].astype(np.float32)
                    + bo for b in range(B)])
    return out.astype(np.float32)


# revision 22
# speedup vs baseline: 1.1576x; 1.0135x over previous
"""Causal multi-head attention block (B=4, S=2048, D=768, H=12, Dh=64)
distributed over 8 NeuronCores: core = (batch, head-group), each core
computes its 6 heads end-to-end plus its partial output projection;
host sums the two partials per batch and adds the bias.

Self-contained: hardcodes all shapes; no sibling imports.
"""

import numpy as np

B, S, D = 4, 2048, 768
H, DH = 12, 64
G = 384          # channels per head group (6 heads)
NPAIR = 3        # head pairs per core
NSC = 4          # 512-wide query windows
W = 512
NST = 16         # 128-row s-tiles
NDC = 6          # 128-row D chunks

# packed-weights layout (per-partition column offsets in the wts tile)
MK0 = 0
QB = [128, 3968, 5504]       # wq pair-column blocks (6 chunks x 128)
KB = [896, 4736, 6272]       # wk pair-column blocks
WV0 = 1664                   # full wv (6 chunks x 384)
WO0 = 7040                   # wo (3 chunks x 768)
WX = 9344

_PROGRAM = None
PROFILE = False
PROFILE_DIR = None
LAST_RESULT = None


def _split_waits(nc, max_waits=1, max_updates=1):
    """This container's walrus rejects instructions carrying more than one
    semaphore wait/update ("Too many sync wait commands").  Move excess
    waits onto NoOps inserted before the owning instruction (same engine)
    and excess updates onto NoOps inserted after."""
    import concourse.mybir as mybir

    counter = [0]

    def nop(engine, waits, updates):
        counter[0] += 1
        n = mybir.InstNoOp(name=f"wsplit_nop_{counter[0]}", ins=[], outs=[])
        n.engine = engine
        n.sync_info = mybir.SyncInfo(on_wait=waits, on_update=updates)
        return n

    for bb in nc.main_func.blocks:
        out = []
        changed = False
        for ins in bb.instructions:
            si = ins.sync_info
            waits = list(si.on_wait) if si and si.on_wait else []
            updates = list(si.on_update) if si and si.on_update else []
            pre, post = [], []
            if len(waits) > max_waits:
                keep = waits[:max_waits - 1] if max_waits > 1 else []
                rest = waits[len(keep):]
                while rest:
                    chunk, rest = rest[:max_waits], rest[max_waits:]
                    pre.append(chunk)
                waits = keep
                changed = True
            if len(updates) > max_updates:
                rest = updates[max_updates:]
                updates = updates[:max_updates]
                while rest:
                    chunk, rest = rest[:max_updates], rest[max_updates:]
                    post.append(chunk)
                changed = True
            if pre or post:
                ins.sync_info = mybir.SyncInfo(
                    on_wait=waits, on_update=updates)
            for w in pre:
                out.append(nop(ins.engine, w, []))
            out.append(ins)
            for u in post:
                out.append(nop(ins.engine, [], u))
        if changed:
            bb.instructions = out


def _install_profile_hooks():
    """Dev-only (PROFILE=True): register the NTFF profile hook that the
    agent image's antenv lacks, and stub out the artifact upload."""
    import sys
    import types

    try:
        from antenv.axon_hooks import get_axon_ntff_profile_hook  # noqa: F401
    except ImportError:
        import antenv
        from trn_agent_boot import trn_boot

        hook = trn_boot._ntff_profile_via_ctypes("/opt/axon/libaxon_pjrt.so")
        mod = types.ModuleType("antenv.axon_hooks")
        mod._hook = hook
        mod.get_axon_ntff_profile_hook = lambda: mod._hook
        mod.set_axon_ntff_profile_hook = lambda h: setattr(mod, "_hook", h)
        sys.modules["antenv.axon_hooks"] = mod
        antenv.axon_hooks = mod

    from concourse import bass_utils

    bass_utils.upload_artifacts = lambda tmpdir: "local://" + tmpdir


def _build_program():
    import concourse.bass as bass
    import concourse.mybir as mybir
    import concourse.tile as tile

    f16 = mybir.dt.float16
    f32 = mybir.dt.float32

    nc = bass.Bass()
    # xt is block-major: [128, window, chunk*512] so each window's slice is
    # one contiguous 6KB-per-partition DMA.
    xt_d = nc.declare_dram_parameter("xt", [128, NSC, NDC * W], f16,
                                     isOutput=False)
    wts_d = nc.declare_dram_parameter("wts", [128, WX], f16, isOutput=False)
    y_d = nc.declare_dram_parameter("y", [S, D], f16, isOutput=True)

    with tile.TileContext(nc) as tc:
        with (
            tc.tile_pool(name="const", bufs=1) as const,
            tc.tile_pool(name="work", bufs=3) as work,
            tc.tile_pool(name="outp", bufs=3) as outp,
            tc.tile_pool(name="ps", bufs=2, space="PSUM") as ps,
        ):
            # ---- persistent SBUF tiles ----
            wts = const.tile([128, WX], f16, name="wts", tag="wts")
            xt = const.tile([128, NSC, NDC * W], f16, name="xt", tag="xt")

            def xv(sc, dc, c0, c1):
                # chunk dc, absolute cols W*sc+c0 .. W*sc+c1
                return xt[:, sc, W * dc + c0:W * dc + c1]
            qt = [const.tile([128, S], f16, name=f"qt{p}", tag=f"qt{p}")
                  for p in range(NPAIR)]
            kt = [const.tile([128, S], f16, name=f"kt{p}", tag=f"kt{p}")
                  for p in range(NPAIR)]
            gt = [const.tile([128, S], f16, name=f"gt{p}", tag=f"gt{p}")
                  for p in range(NPAIR)]
            # vt[st]: per head h the 128 lhsT columns [v_h (64) | ones (64)]
            # so one matmul per head accumulates attn@V on out partitions
            # 0:64 and the softmax denominator (replicated) on 64:128.
            vt = [const.tile([128, 2 * NPAIR, 128], f16, name=f"vt{t}",
                             tag=f"vt{t}") for t in range(NST)]

            mkv = wts[:, MK0:MK0 + 128]

            def wqv(pair, dc):
                return wts[:, QB[pair] + 128 * dc:QB[pair] + 128 * (dc + 1)]

            def wkv(pair, dc):
                return wts[:, KB[pair] + 128 * dc:KB[pair] + 128 * (dc + 1)]

            def wvv(dc):
                return wts[:, WV0 + 384 * dc:WV0 + 384 * (dc + 1)]

            def wov(cc, half):
                b = WO0 + 768 * cc + 384 * half
                return wts[:, b:b + 384]

            # ---- input DMAs, need-ordered; both queues share one HBM
            # stream so the first-window deps (mk+pair0 qk, xt cols 0:512,
            # wv) go first and the rest rides behind compute ----
            # single queue, exact need order: the HBM stream is shared, so
            # interleaving a second queue only delays the critical set.
            # y-output DMAs ride the gpsimd queue instead.
            nc.sync.dma_start(out=wts[:, 0:WV0], in_=wts_d[:, 0:WV0])
            nc.sync.dma_start(out=xt[:, 0, :], in_=xt_d[:, 0, :])
            nc.sync.dma_start(out=wts[:, WV0:QB[1]],
                              in_=wts_d[:, WV0:QB[1]])
            nc.sync.dma_start(out=wts[:, QB[1]:WO0],
                              in_=wts_d[:, QB[1]:WO0])
            nc.sync.dma_start(out=xt[:, 1, :], in_=xt_d[:, 1, :])
            nc.sync.dma_start(out=xt[:, 2, :], in_=xt_d[:, 2, :])
            nc.sync.dma_start(out=xt[:, 3, :], in_=xt_d[:, 3, :])
            nc.sync.dma_start(out=wts[:, WO0:WX], in_=wts_d[:, WO0:WX])

            for st in range(NST):
                nc.vector.memset(vt[st][:, :, 64:128], 1.0)

            def proj_qk_unit(pair, sc):
                qp = ps.tile([128, W], f32, name=f"qp{pair}_{sc}",
                             tag="sc", bufs=2)
                for dc in range(NDC):
                    nc.tensor.matmul(
                        qp, wqv(pair, dc), xv(sc, dc, 0, W),
                        start=(dc == 0), stop=(dc == NDC - 1))
                nc.vector.tensor_copy(
                    out=qt[pair][:, W * sc:W * (sc + 1)], in_=qp)
                kp = ps.tile([128, W], f32, name=f"kp{pair}_{sc}",
                             tag="sc", bufs=2)
                for dc in range(NDC):
                    nc.tensor.matmul(
                        kp, wkv(pair, dc), xv(sc, dc, 0, W),
                        start=(dc == 0), stop=(dc == NDC - 1))
                nc.vector.tensor_copy(
                    out=kt[pair][:, W * sc:W * (sc + 1)], in_=kp)

            def proj_v(st):
                vp = ps.tile([128, 2 * NPAIR, 64], f32, name=f"vp{st}",
                             tag="sc", bufs=2)
                for dc in range(NDC):
                    nc.tensor.matmul(
                        vp,
                        xv(st // 4, dc, 128 * (st % 4), 128 * (st % 4 + 1)),
                        wvv(dc),
                        start=(dc == 0), stop=(dc == NDC - 1))
                nc.vector.tensor_copy(out=vt[st][:, :, 0:64], in_=vp)

            def outproj(st):
                o0 = ps.tile([128, G], f32, name=f"o0_{st}", tag="sc",
                             bufs=2)
                for cc in range(3):
                    nc.tensor.matmul(
                        o0, gt[cc][:, 128 * st:128 * (st + 1)], wov(cc, 0),
                        start=(cc == 0), stop=(cc == 2))
                o1 = ps.tile([128, G], f32, name=f"o1_{st}", tag="sc",
                             bufs=2)
                for cc in range(3):
                    nc.tensor.matmul(
                        o1, gt[cc][:, 128 * st:128 * (st + 1)], wov(cc, 1),
                        start=(cc == 0), stop=(cc == 2))
                ob = outp.tile([128, D], f16, name=f"ob{st}", tag="ob",
                               bufs=4)
                nc.vector.tensor_copy(out=ob[:, 0:G], in_=o0)
                nc.vector.tensor_copy(out=ob[:, G:D], in_=o1)
                nc.gpsimd.dma_start(
                    out=y_d[128 * st:128 * (st + 1), :], in_=ob)

            # ---- attention: one global software pipeline over all
            # (sc, pair, jb) blocks so neither the PE nor ACT drains at
            # window or pair boundaries ----
            def scores_exp(pair, sc, jb):
                col0 = max(0, 128 * jb - W * sc)
                sct = ps.tile([128, 1024], f32, name=f"sc{pair}_{sc}_{jb}",
                              tag="sc", bufs=2)
                nc.tensor.matmul(
                    sct[:, col0:W],
                    kt[pair][0:64, 128 * jb:128 * (jb + 1)],
                    qt[pair][0:64, W * sc + col0:W * (sc + 1)],
                    start=True, stop=True)
                nc.tensor.matmul(
                    sct[:, W:2 * W - col0],
                    kt[pair][64:128, 128 * jb:128 * (jb + 1)],
                    qt[pair][64:128, W * sc + col0:W * (sc + 1)],
                    start=True, stop=True)
                ex = work.tile([128, 1024], f16, name=f"ex{pair}_{sc}_{jb}",
                               tag="exp", bufs=6)
                nc.scalar.activation(
                    out=ex[:, col0:2 * W - col0],
                    in_=sct[:, col0:2 * W - col0],
                    func=mybir.ActivationFunctionType.Exp, scale=0.125)
                if jb >= 4 * sc:  # zero the j>i triangle of the diag block
                    nc.gpsimd.tensor_mul(
                        ex[:, col0:col0 + 128], ex[:, col0:col0 + 128], mkv)
                    nc.gpsimd.tensor_mul(
                        ex[:, W:W + 128], ex[:, W:W + 128], mkv)
                return ex

            def finalize_cols(pair, sc, q0, q1, pv0, pv1):
                # Normalize query cols [q0:q1) of this window into gt.
                # 1/dn as exp(-ln(dn)) on ScalarE: ln+exp share one
                # activation table set, so no table thrash, and the DVE
                # FIFO stays clear of the slow iterative reciprocal.
                w = q1 - q0
                cols = slice(W * sc + q0, W * sc + q1)
                dnb = work.tile([128, w], f32, name=f"dn{pair}_{sc}_{q0}",
                                tag="dnb", bufs=2)
                nc.vector.tensor_copy(out=dnb[0:64, :],
                                      in_=pv0[64:128, q0:q1])
                nc.vector.tensor_copy(out=dnb[64:128, :],
                                      in_=pv1[64:128, q0:q1])
                rc = work.tile([128, w], f32, name=f"rc{pair}_{sc}_{q0}",
                               tag="rc", bufs=2)
                nc.scalar.activation(
                    out=rc, in_=dnb,
                    func=mybir.ActivationFunctionType.Ln)
                nc.scalar.activation(
                    out=rc, in_=rc,
                    func=mybir.ActivationFunctionType.Exp, scale=-1.0)
                nc.vector.tensor_mul(
                    gt[pair][0:64, cols], pv0[0:64, q0:q1], rc[0:64, :])
                nc.vector.tensor_mul(
                    gt[pair][64:128, cols], pv1[0:64, q0:q1], rc[64:128, :])

            LASTWIN = (NPAIR - 1, NSC - 1)

            def pv_dn(state):
                pv0, pv1, pair, sc, jb, ex = state
                col0 = max(0, 128 * jb - W * sc)
                first, last = (jb == 0), (jb == 4 * sc + 3)
                nc.tensor.matmul(
                    pv0[:, col0:W], vt[jb][:, 2 * pair, :],
                    ex[:, col0:W], start=first, stop=last)
                nc.tensor.matmul(
                    pv1[:, col0:W], vt[jb][:, 2 * pair + 1, :],
                    ex[:, W:2 * W - col0], start=first, stop=last)
                if (pair, sc) == LASTWIN and jb >= 4 * sc:
                    # last window: strip c of the diagonal is complete after
                    # block jb=4*sc+c (later blocks only write cols >=128*
                    # (c+1)), so normalize + out-project strip-by-strip to
                    # keep the PE busy through the tail.
                    c = jb - 4 * sc
                    finalize_cols(pair, sc, 128 * c, 128 * (c + 1), pv0, pv1)
                    outproj(4 * sc + c)
                elif last:  # window complete: normalize into gt
                    finalize_cols(pair, sc, 0, W, pv0, pv1)

            # static filler plan: emit projection / out-proj units after
            # given global block indices (they're needed ~one round later
            # than emitted; DMA arrival order matches)
            fillers = {
                0: [lambda: proj_v(1)],
                1: [lambda: proj_qk_unit(1, 0)],
                2: [lambda: proj_v(2)],
                3: [lambda: proj_v(3)],
                4: [lambda: proj_qk_unit(2, 0)],
                6: [lambda: proj_qk_unit(0, 1)],
                8: [lambda: proj_qk_unit(1, 1)],
                10: [lambda: proj_qk_unit(2, 1)],
                12: [lambda: proj_v(4)],
                14: [lambda: proj_v(5)],
                16: [lambda: proj_v(6)],
                18: [lambda: proj_v(7)],
                20: [lambda: outproj(0)],
                22: [lambda: outproj(1)],
                24: [lambda: outproj(2)],
                26: [lambda: outproj(3)],
                28: [lambda: proj_qk_unit(0, 2)],
                30: [lambda: proj_qk_unit(1, 2)],
                32: [lambda: proj_qk_unit(2, 2)],
                36: [lambda: proj_v(8)],
                38: [lambda: proj_v(9)],
                40: [lambda: proj_v(10)],
                42: [lambda: proj_v(11)],
                45: [lambda: outproj(4)],
                48: [lambda: outproj(5)],
                51: [lambda: outproj(6)],
                54: [lambda: outproj(7)],
                57: [lambda: proj_qk_unit(0, 3)],
                60: [lambda: proj_qk_unit(1, 3)],
                63: [lambda: proj_qk_unit(2, 3)],
                66: [lambda: proj_v(12)],
                68: [lambda: proj_v(13)],
                70: [lambda: proj_v(14)],
                72: [lambda: proj_v(15)],
                75: [lambda: outproj(8)],
                79: [lambda: outproj(9)],
                83: [lambda: outproj(10)],
                87: [lambda: outproj(11)],
            }

            proj_qk_unit(0, 0)
            proj_v(0)

            prev = [None]

            def block(pair, sc, jb, pv0, pv1):
                ex = scores_exp(pair, sc, jb)
                if prev[0] is not None:
                    pv_dn(prev[0])
                prev[0] = (pv0, pv1, pair, sc, jb, ex)

            g = 0
            for sc in range(NSC):
                for pair in range(NPAIR):
                    pv0 = ps.tile([128, W], f32, name=f"pv0_{pair}_{sc}",
                                  tag="apv", bufs=2)
                    pv1 = ps.tile([128, W], f32, name=f"pv1_{pair}_{sc}",
                                  tag="adn", bufs=2)
                    for jb in range(4 * sc + 4):
                        block(pair, sc, jb, pv0, pv1)
                        for fn in fillers.get(g, ()):
                            fn()
                        g += 1
            pv_dn(prev[0])

    _split_waits(nc)
    return nc


def _get_program():
    global _PROGRAM
    if _PROGRAM is None:
        _PROGRAM = _build_program()
    return _PROGRAM


def _pack_chunks(wT, width):
    # [768, width] -> [128, 6*width] with chunk-major per-partition layout
    return np.ascontiguousarray(
        wT.reshape(NDC, 128, width).transpose(1, 0, 2).reshape(128, -1))


def kernel(x, Wq, Wk, Wv, Wo, bo):
    global LAST_RESULT
    from concourse.bass_utils import run_bass_kernel_spmd

    x = np.asarray(x, np.float32)
    Wq = np.asarray(Wq, np.float32)
    Wk = np.asarray(Wk, np.float32)
    Wv = np.asarray(Wv, np.float32)
    Wo = np.asarray(Wo, np.float32)
    bo = np.asarray(bo, np.float32)

    tri = np.tril(np.ones((128, 128), np.float32)).T  # 1 where j<=i
    mk = tri.astype(np.float16)

    in_maps = []
    for c in range(8):
        b, gi = divmod(c, 2)
        hs = slice(G * gi, G * (gi + 1))
        xt = np.ascontiguousarray(x[b].T).astype(np.float16)
        xt3 = xt.reshape(NDC, 128, NSC, W).transpose(1, 2, 0, 3).reshape(
            128, NSC, NDC * W)
        wqT = Wq[hs, :].T.astype(np.float16)   # [768, 384]
        wkT = Wk[hs, :].T.astype(np.float16)
        wvT = Wv[hs, :].T.astype(np.float16)
        woT = Wo[:, hs].T.astype(np.float16)   # [384, 768]

        parts = [mk]
        for pr in range(NPAIR):
            if pr == 1:
                parts.append(_pack_chunks(wvT, G))
            parts.append(_pack_chunks(
                np.ascontiguousarray(wqT[:, 128 * pr:128 * (pr + 1)]), 128))
            parts.append(_pack_chunks(
                np.ascontiguousarray(wkT[:, 128 * pr:128 * (pr + 1)]), 128))
        parts.append(np.ascontiguousarray(
            woT.reshape(3, 128, D).transpose(1, 0, 2).reshape(128, -1)))
        wts = np.concatenate(parts, axis=1)
        assert wts.shape == (128, WX), wts.shape

        in_maps.append({
            "xt": np.ascontiguousarray(xt3),
            "wts": np.ascontiguousarray(wts),
        })

    if PROFILE:
        _install_profile_hooks()
    nc = _get_program()
    res = run_bass_kernel_spmd(nc, in_maps, core_ids=list(range(8)),
                               trace=PROFILE, tmpdir=PROFILE_DIR)
    LAST_RESULT = res
    parts = [res.results[c]["y"] for c in range(8)]
    out = np.stack([parts[2 * b].astype(np.float32)
                    + parts[2 * b + 1].astype(np.float32)
                    + bo for b in range(B)])
    return out.astype(np.float32)


# revision 33
# speedup vs baseline: 1.1874x; 1.0257x over previous
"""Causal multi-head attention block (B=4, S=2048, D=768, H=12, Dh=64)
distributed over 8 NeuronCores: core = (batch, head-group), each core
computes its 6 heads end-to-end plus its partial output projection;
host sums the two partials per batch and adds the bias.

Self-contained: hardcodes all shapes; no sibling imports.
"""

import numpy as np

B, S, D = 4, 2048, 768
H, DH = 12, 64
G = 384          # channels per head group (6 heads)
NPAIR = 3        # head pairs per core
NSC = 4          # 512-wide query windows
W = 512
NST = 16         # 128-row s-tiles
NDC = 6          # 128-row D chunks

# packed-weights layout (per-partition column offsets in the wts tile)
MK0 = 0
WV0 = 128                    # full wv (6 chunks x 384)
WO0 = 2432                   # wo (3 chunks x 768)
WX = 4736
SC8 = 32.0                   # fp8 q/k weight pre-scale (avoids e4m3
                             # subnormals at w~0.02); undone in exp scale

_PROGRAM = None
PROFILE = False
PROFILE_DIR = None
LAST_RESULT = None


def _split_waits(nc, max_waits=1, max_updates=1):
    """This container's walrus rejects instructions carrying more than one
    semaphore wait/update ("Too many sync wait commands").  Move excess
    waits onto NoOps inserted before the owning instruction (same engine)
    and excess updates onto NoOps inserted after."""
    import concourse.mybir as mybir

    counter = [0]

    def nop(engine, waits, updates):
        counter[0] += 1
        n = mybir.InstNoOp(name=f"wsplit_nop_{counter[0]}", ins=[], outs=[])
        n.engine = engine
        n.sync_info = mybir.SyncInfo(on_wait=waits, on_update=updates)
        return n

    for bb in nc.main_func.blocks:
        out = []
        changed = False
        for ins in bb.instructions:
            si = ins.sync_info
            waits = list(si.on_wait) if si and si.on_wait else []
            updates = list(si.on_update) if si and si.on_update else []
            pre, post = [], []
            if len(waits) > max_waits:
                keep = waits[:max_waits - 1] if max_waits > 1 else []
                rest = waits[len(keep):]
                while rest:
                    chunk, rest = rest[:max_waits], rest[max_waits:]
                    pre.append(chunk)
                waits = keep
                changed = True
            if len(updates) > max_updates:
                rest = updates[max_updates:]
                updates = updates[:max_updates]
                while rest:
                    chunk, rest = rest[:max_updates], rest[max_updates:]
                    post.append(chunk)
                changed = True
            if pre or post:
                ins.sync_info = mybir.SyncInfo(
                    on_wait=waits, on_update=updates)
            for w in pre:
                out.append(nop(ins.engine, w, []))
            out.append(ins)
            for u in post:
                out.append(nop(ins.engine, [], u))
        if changed:
            bb.instructions = out


def _install_profile_hooks():
    """Dev-only (PROFILE=True): register the NTFF profile hook that the
    agent image's antenv lacks, and stub out the artifact upload."""
    import sys
    import types

    try:
        from antenv.axon_hooks import get_axon_ntff_profile_hook  # noqa: F401
    except ImportError:
        import antenv
        from trn_agent_boot import trn_boot

        hook = trn_boot._ntff_profile_via_ctypes("/opt/axon/libaxon_pjrt.so")
        mod = types.ModuleType("antenv.axon_hooks")
        mod._hook = hook
        mod.get_axon_ntff_profile_hook = lambda: mod._hook
        mod.set_axon_ntff_profile_hook = lambda h: setattr(mod, "_hook", h)
        sys.modules["antenv.axon_hooks"] = mod
        antenv.axon_hooks = mod

    from concourse import bass_utils

    bass_utils.upload_artifacts = lambda tmpdir: "local://" + tmpdir


def _build_program():
    import concourse.bass as bass
    import concourse.mybir as mybir
    import concourse.tile as tile

    f16 = mybir.dt.float16
    f32 = mybir.dt.float32
    f8 = mybir.dt.float8e4

    nc = bass.Bass()
    # xt is block-major: [128, window, chunk*512] so each window's slice is
    # one contiguous 6KB-per-partition DMA.  xt8/wqk8 are fp8 copies used
    # only by the DoubleRow q/k projections.
    xt_d = nc.declare_dram_parameter("xt", [128, NSC, NDC * W], f16,
                                     isOutput=False)
    xt8_d = nc.declare_dram_parameter("xt8", [128, NSC, NDC, W], f8,
                                      isOutput=False)
    wqk8_d = nc.declare_dram_parameter("wqk8", [128, 2 * NPAIR, NDC, 128],
                                       f8, isOutput=False)
    wts_d = nc.declare_dram_parameter("wts", [128, WX], f16, isOutput=False)
    y_d = nc.declare_dram_parameter("y", [S, D], f16, isOutput=True)

    with tile.TileContext(nc) as tc:
        with (
            tc.tile_pool(name="const", bufs=1) as const,
            tc.tile_pool(name="work", bufs=3) as work,
            tc.tile_pool(name="outp", bufs=3) as outp,
            tc.tile_pool(name="ps", bufs=2, space="PSUM") as ps,
        ):
            # ---- persistent SBUF tiles ----
            wts = const.tile([128, WX], f16, name="wts", tag="wts")
            xt = const.tile([128, NSC, NDC * W], f16, name="xt", tag="xt")
            xt8 = const.tile([128, NSC, NDC, W], f8, name="xt8", tag="xt8")
            wqk8 = const.tile([128, 2 * NPAIR, NDC, 128], f8, name="wqk8",
                              tag="wqk8")

            def xv(sc, dc, c0, c1):
                # chunk dc, absolute cols W*sc+c0 .. W*sc+c1
                return xt[:, sc, W * dc + c0:W * dc + c1]
            qt = [const.tile([128, S], f16, name=f"qt{p}", tag=f"qt{p}")
                  for p in range(NPAIR)]
            kt = [const.tile([128, S], f16, name=f"kt{p}", tag=f"kt{p}")
                  for p in range(NPAIR)]
            gt = [const.tile([128, S], f16, name=f"gt{p}", tag=f"gt{p}")
                  for p in range(NPAIR)]
            # vt[st]: per head h the 128 lhsT columns [v_h (64) | ones (64)]
            # so one matmul per head accumulates attn@V on out partitions
            # 0:64 and the softmax denominator (replicated) on 64:128.
            vt = [const.tile([128, 2 * NPAIR, 128], f16, name=f"vt{t}",
                             tag=f"vt{t}") for t in range(NST)]

            mkv = wts[:, MK0:MK0 + 128]

            def wvv(dc):
                return wts[:, WV0 + 384 * dc:WV0 + 384 * (dc + 1)]

            def wov(cc, half):
                b = WO0 + 768 * cc + 384 * half
                return wts[:, b:b + 384]

            # ---- input DMAs, need-ordered; both queues share one HBM
            # stream so the first-window deps (mk+pair0 qk, xt cols 0:512,
            # wv) go first and the rest rides behind compute ----
            # single queue, exact need order: the HBM stream is shared, so
            # interleaving a second queue only delays the critical set.
            # y-output DMAs ride the gpsimd queue instead.
            nc.sync.dma_start(out=wts[:, 0:WV0], in_=wts_d[:, 0:WV0])  # mk
            nc.sync.dma_start(out=wqk8[:, 0:2, :, :],
                              in_=wqk8_d[:, 0:2, :, :])
            nc.sync.dma_start(out=xt8[:, 0, :, :], in_=xt8_d[:, 0, :, :])
            nc.sync.dma_start(out=wts[:, WV0:WO0], in_=wts_d[:, WV0:WO0])
            nc.sync.dma_start(out=xt[:, 0, :], in_=xt_d[:, 0, :])
            nc.sync.dma_start(out=wqk8[:, 2:6, :, :],
                              in_=wqk8_d[:, 2:6, :, :])
            nc.sync.dma_start(out=xt8[:, 1, :, :], in_=xt8_d[:, 1, :, :])
            nc.sync.dma_start(out=xt[:, 1, :], in_=xt_d[:, 1, :])
            nc.sync.dma_start(out=xt8[:, 2, :, :], in_=xt8_d[:, 2, :, :])
            nc.sync.dma_start(out=xt[:, 2, :], in_=xt_d[:, 2, :])
            nc.sync.dma_start(out=xt8[:, 3, :, :], in_=xt8_d[:, 3, :, :])
            nc.sync.dma_start(out=xt[:, 3, :], in_=xt_d[:, 3, :])
            nc.sync.dma_start(out=wts[:, WO0:WX], in_=wts_d[:, WO0:WX])

            for st in range(NST):
                nc.vector.memset(vt[st][:, :, 64:128], 1.0)

            def proj_qk_unit(pair, sc):
                # fp8 DoubleRow: 3 matmuls of 2 packed 128-chunks each
                DR = mybir.MatmulPerfMode.DoubleRow
                qp = ps.tile([128, W], f32, name=f"qp{pair}_{sc}",
                             tag="sc", bufs=2)
                for g2 in range(3):
                    nc.tensor.matmul(
                        qp, wqk8[:, 2 * pair, 2 * g2:2 * g2 + 2, :],
                        xt8[:, sc, 2 * g2:2 * g2 + 2, :],
                        start=(g2 == 0), stop=(g2 == 2), perf_mode=DR)
                nc.vector.tensor_copy(
                    out=qt[pair][:, W * sc:W * (sc + 1)], in_=qp)
                kp = ps.tile([128, W], f32, name=f"kp{pair}_{sc}",
                             tag="sc", bufs=2)
                for g2 in range(3):
                    nc.tensor.matmul(
                        kp, wqk8[:, 2 * pair + 1, 2 * g2:2 * g2 + 2, :],
                        xt8[:, sc, 2 * g2:2 * g2 + 2, :],
                        start=(g2 == 0), stop=(g2 == 2), perf_mode=DR)
                nc.vector.tensor_copy(
                    out=kt[pair][:, W * sc:W * (sc + 1)], in_=kp)

            def proj_v(st):
                vp = ps.tile([128, 2 * NPAIR, 64], f32, name=f"vp{st}",
                             tag="sc", bufs=2)
                for dc in range(NDC):
                    nc.tensor.matmul(
                        vp,
                        xv(st // 4, dc, 128 * (st % 4), 128 * (st % 4 + 1)),
                        wvv(dc),
                        start=(dc == 0), stop=(dc == NDC - 1))
                nc.vector.tensor_copy(out=vt[st][:, :, 0:64], in_=vp)

            def outproj(st):
                o0 = ps.tile([128, G], f32, name=f"o0_{st}", tag="sc",
                             bufs=2)
                for cc in range(3):
                    nc.tensor.matmul(
                        o0, gt[cc][:, 128 * st:128 * (st + 1)], wov(cc, 0),
                        start=(cc == 0), stop=(cc == 2))
                o1 = ps.tile([128, G], f32, name=f"o1_{st}", tag="sc",
                             bufs=2)
                for cc in range(3):
                    nc.tensor.matmul(
                        o1, gt[cc][:, 128 * st:128 * (st + 1)], wov(cc, 1),
                        start=(cc == 0), stop=(cc == 2))
                ob = outp.tile([128, D], f16, name=f"ob{st}", tag="ob",
                               bufs=4)
                nc.vector.tensor_copy(out=ob[:, 0:G], in_=o0)
                nc.vector.tensor_copy(out=ob[:, G:D], in_=o1)
                nc.gpsimd.dma_start(
                    out=y_d[128 * st:128 * (st + 1), :], in_=ob)

            # ---- attention: one global software pipeline over all
            # (sc, pair, jb) blocks so neither the PE nor ACT drains at
            # window or pair boundaries ----
            def scores_exp(pair, sc, jb):
                col0 = max(0, 128 * jb - W * sc)
                sct = ps.tile([128, 1024], f32, name=f"sc{pair}_{sc}_{jb}",
                              tag="sc", bufs=2)
                nc.tensor.matmul(
                    sct[:, col0:W],
                    kt[pair][0:64, 128 * jb:128 * (jb + 1)],
                    qt[pair][0:64, W * sc + col0:W * (sc + 1)],
                    start=True, stop=True)
                nc.tensor.matmul(
                    sct[:, W:2 * W - col0],
                    kt[pair][64:128, 128 * jb:128 * (jb + 1)],
                    qt[pair][64:128, W * sc + col0:W * (sc + 1)],
                    start=True, stop=True)
                ex = work.tile([128, 1024], f16, name=f"ex{pair}_{sc}_{jb}",
                               tag="exp", bufs=6)
                nc.scalar.activation(
                    out=ex[:, col0:2 * W - col0],
                    in_=sct[:, col0:2 * W - col0],
                    func=mybir.ActivationFunctionType.Exp,
                    scale=0.125 / (SC8 * SC8))
                if jb >= 4 * sc:  # zero the j>i triangle of the diag block
                    nc.gpsimd.tensor_mul(
                        ex[:, col0:col0 + 128], ex[:, col0:col0 + 128], mkv)
                    nc.gpsimd.tensor_mul(
                        ex[:, W:W + 128], ex[:, W:W + 128], mkv)
                return ex

            def finalize_copy(pair, sc, q0, q1, pv0, pv1):
                w = q1 - q0
                dnb = work.tile([128, w], f32, name=f"dn{pair}_{sc}_{q0}",
                                tag="dnb", bufs=2)
                nc.vector.tensor_copy(out=dnb[0:64, :],
                                      in_=pv0[64:128, q0:q1])
                nc.vector.tensor_copy(out=dnb[64:128, :],
                                      in_=pv1[64:128, q0:q1])
                return dnb

            def finalize_norm(pair, sc, q0, q1, pv0, pv1, dnb):
                # Normalize query cols [q0:q1) of this window into gt.
                # 1/dn as exp(-ln(dn)) on ScalarE: ln+exp share one
                # activation table set, so no table thrash, and the DVE
                # FIFO stays clear of the slow iterative reciprocal.
                w = q1 - q0
                cols = slice(W * sc + q0, W * sc + q1)
                rc = work.tile([128, w], f32, name=f"rc{pair}_{sc}_{q0}",
                               tag="rc", bufs=2)
                nc.scalar.activation(
                    out=rc, in_=dnb,
                    func=mybir.ActivationFunctionType.Ln)
                nc.scalar.activation(
                    out=rc, in_=rc,
                    func=mybir.ActivationFunctionType.Exp, scale=-1.0)
                nc.vector.tensor_mul(
                    gt[pair][0:64, cols], pv0[0:64, q0:q1], rc[0:64, :])
                nc.vector.tensor_mul(
                    gt[pair][64:128, cols], pv1[0:64, q0:q1], rc[64:128, :])

            LASTWIN = (NPAIR - 1, NSC - 1)

            def pv_dn(state):
                pv0, pv1, pair, sc, jb, ex = state
                col0 = max(0, 128 * jb - W * sc)
                first, last = (jb == 0), (jb == 4 * sc + 3)
                nc.tensor.matmul(
                    pv0[:, col0:W], vt[jb][:, 2 * pair, :],
                    ex[:, col0:W], start=first, stop=last)
                nc.tensor.matmul(
                    pv1[:, col0:W], vt[jb][:, 2 * pair + 1, :],
                    ex[:, W:2 * W - col0], start=first, stop=last)
                if (pair, sc) == LASTWIN and jb >= 4 * sc:
                    # last window: strip c of the diagonal is complete after
                    # block jb=4*sc+c (later blocks only write cols >=128*
                    # (c+1)), so normalize + out-project strip-by-strip to
                    # keep the PE busy through the tail.
                    c = jb - 4 * sc
                    dnb = finalize_copy(pair, sc, 128 * c, 128 * (c + 1),
                                        pv0, pv1)
                    finalize_norm(pair, sc, 128 * c, 128 * (c + 1),
                                  pv0, pv1, dnb)
                    outproj(4 * sc + c)
                elif last:
                    # copy dn out now; defer ln/exp+muls ~2 blocks so
                    # neither the ACT nor DVE FIFO idle-waits on the other
                    dnb = finalize_copy(pair, sc, 0, W, pv0, pv1)
                    pending.append((gcur[0] + 2, lambda p=pair, s=sc,
                                    a=pv0, b=pv1, d=dnb:
                                    finalize_norm(p, s, 0, W, a, b, d)))

            # static filler plan: emit projection / out-proj units after
            # given global block indices (they're needed ~one round later
            # than emitted; DMA arrival order matches)
            fillers = {
                0: [lambda: proj_v(1)],
                1: [lambda: proj_qk_unit(1, 0)],
                2: [lambda: proj_v(2)],
                3: [lambda: proj_v(3)],
                4: [lambda: proj_qk_unit(2, 0)],
                6: [lambda: proj_qk_unit(0, 1)],
                8: [lambda: proj_qk_unit(1, 1)],
                10: [lambda: proj_qk_unit(2, 1)],
                12: [lambda: proj_v(4)],
                14: [lambda: proj_v(5)],
                16: [lambda: proj_v(6)],
                18: [lambda: proj_v(7)],
                20: [lambda: outproj(0)],
                22: [lambda: outproj(1)],
                24: [lambda: outproj(2)],
                26: [lambda: outproj(3)],
                28: [lambda: proj_qk_unit(0, 2)],
                30: [lambda: proj_qk_unit(1, 2)],
                32: [lambda: proj_qk_unit(2, 2)],
                36: [lambda: proj_v(8)],
                38: [lambda: proj_v(9)],
                40: [lambda: proj_v(10)],
                42: [lambda: proj_v(11)],
                45: [lambda: outproj(4)],
                48: [lambda: outproj(5)],
                51: [lambda: outproj(6)],
                54: [lambda: outproj(7)],
                57: [lambda: proj_qk_unit(0, 3)],
                60: [lambda: proj_qk_unit(1, 3)],
                63: [lambda: proj_qk_unit(2, 3)],
                66: [lambda: proj_v(12)],
                68: [lambda: proj_v(13)],
                70: [lambda: proj_v(14)],
                72: [lambda: proj_v(15)],
                75: [lambda: outproj(8)],
                79: [lambda: outproj(9)],
                83: [lambda: outproj(10)],
                87: [lambda: outproj(11)],
            }

            proj_qk_unit(0, 0)
            proj_v(0)

            prev = [None]
            pending = []
            gcur = [0]

            def block(pair, sc, jb, pv0, pv1):
                ex = scores_exp(pair, sc, jb)
                if prev[0] is not None:
                    pv_dn(prev[0])
                prev[0] = (pv0, pv1, pair, sc, jb, ex)

            for sc in range(NSC):
                for pair in range(NPAIR):
                    pv0 = ps.tile([128, W], f32, name=f"pv0_{pair}_{sc}",
                                  tag="apv", bufs=2)
                    pv1 = ps.tile([128, W], f32, name=f"pv1_{pair}_{sc}",
                                  tag="adn", bufs=2)
                    for jb in range(4 * sc + 4):
                        block(pair, sc, jb, pv0, pv1)
                        while pending and pending[0][0] <= gcur[0]:
                            pending.pop(0)[1]()
                        for fn in fillers.get(gcur[0], ()):
                            fn()
                        gcur[0] += 1
            pv_dn(prev[0])
            for _, fn in pending:
                fn()

    _split_waits(nc)
    return nc


def _get_program():
    global _PROGRAM
    if _PROGRAM is None:
        _PROGRAM = _build_program()
    return _PROGRAM


def _pack_chunks(wT, width):
    # [768, width] -> [128, 6*width] with chunk-major per-partition layout
    return np.ascontiguousarray(
        wT.reshape(NDC, 128, width).transpose(1, 0, 2).reshape(128, -1))


def kernel(x, Wq, Wk, Wv, Wo, bo):
    global LAST_RESULT
    from concourse.bass_utils import run_bass_kernel_spmd

    x = np.asarray(x, np.float32)
    Wq = np.asarray(Wq, np.float32)
    Wk = np.asarray(Wk, np.float32)
    Wv = np.asarray(Wv, np.float32)
    Wo = np.asarray(Wo, np.float32)
    bo = np.asarray(bo, np.float32)

    tri = np.tril(np.ones((128, 128), np.float32)).T  # 1 where j<=i
    mk = tri.astype(np.float16)

    in_maps = []
    for c in range(8):
        b, gi = divmod(c, 2)
        hs = slice(G * gi, G * (gi + 1))
        import ml_dtypes
        f8 = ml_dtypes.float8_e4m3

        xt = np.ascontiguousarray(x[b].T).astype(np.float16)
        xt3 = xt.reshape(NDC, 128, NSC, W).transpose(1, 2, 0, 3).reshape(
            128, NSC, NDC * W)
        xt8 = np.ascontiguousarray(
            xt3.reshape(128, NSC, NDC, W)).astype(f8)
        wqT = Wq[hs, :].T.astype(np.float32)   # [768, 384]
        wkT = Wk[hs, :].T.astype(np.float32)
        wvT = Wv[hs, :].T.astype(np.float16)
        woT = Wo[:, hs].T.astype(np.float16)   # [384, 768]

        wqk8 = np.zeros((128, 2 * NPAIR, NDC, 128), f8)
        for pr in range(NPAIR):
            for t, wT in ((0, wqT), (1, wkT)):
                wqk8[:, 2 * pr + t] = (
                    wT[:, 128 * pr:128 * (pr + 1)] * SC8
                ).reshape(NDC, 128, 128).transpose(1, 0, 2).astype(f8)

        wts = np.concatenate([
            mk,
            _pack_chunks(wvT, G),
            np.ascontiguousarray(
                woT.reshape(3, 128, D).transpose(1, 0, 2).reshape(128, -1)),
        ], axis=1)
        assert wts.shape == (128, WX), wts.shape

        in_maps.append({
            "xt": np.ascontiguousarray(xt3),
            "xt8": xt8,
            "wqk8": np.ascontiguousarray(wqk8),
            "wts": np.ascontiguousarray(wts),
        })

    if PROFILE:
        _install_profile_hooks()
    nc = _get_program()
    res = run_bass_kernel_spmd(nc, in_maps, core_ids=list(range(8)),
                               trace=PROFILE, tmpdir=PROFILE_DIR)
    LAST_RESULT = res
    parts = [res.results[c]["y"] for c in range(8)]
    out = np.stack([parts[2 * b].astype(np.float32)
                    + parts[2 * b + 1].astype(np.float32)
                    + bo for b in range(B)])
    return out.astype(np.float32)
